# revision 50
# baseline (speedup 1.0000x reference)
"""Multi-head attention (b=2, sq=skv=2048, dim=1024, 16 heads x 64) on 8 TRN2
NeuronCores.

Sharding: 2 heads per core (head-parallel across batch*heads), with the
matching tensor-parallel column slice of W_qkv and row slice of W_out.  Each
core computes a partial output projection over its 128 head-dims; the
all-reduce of the 8 partials (+ bias) happens on the host during unshard.

Per-core kernel (bf16 compute, fp32 PSUM accumulation):
  phase 1: qT/kT/vT = W.T @ x.T   ([128 = 2 heads x 64 dims, tokens]); v is
           additionally PE-transposed to natural [token, dim] layout with a
           ones column appended (denominator trick).
  phase 2: per (batch, q-tile, k-tile): scoresT for both heads ([k-tokens, q])
           in one 2-bank PSUM group; one exp ACTIVATE over the group (scale
           1/8 fused, no max subtraction -- scores range +-10); PV matmuls
           accumulate [v | 1].T @ expT over the 16 k-tiles giving unnormalized
           outT plus the softmax denominator in row 64.  The accumulator is
           copied to SBUF immediately (releasing PSUM); normalization
           (reciprocal + PE outer-product broadcast + multiply) happens off
           the critical path.
  phase 3: partial out = outT.T @ W_out_rows -> bf16 [tokens, 1024].

Emission is orchestrated so the dependency-driven Tile scheduler always has
filler PE work (batch-1 projections, out-projection quarters) inside the
ACT(exp)-bound attention stream, keeping the PE HAM-warm.
"""

import os
import sys

for _p in ("/opt/trn_rl_repo", "/root/.axon_site/_ro/trn_rl_repo"):
    if os.path.isdir(_p) and _p not in sys.path:
        sys.path.append(_p)

import ml_dtypes
import numpy as np

import concourse.bass as bass  # noqa: F401
import concourse.tile as tile
from concourse import bacc, mybir
from concourse.bass_utils import run_bass_kernel_spmd
from concourse.masks import make_identity

B, SQ, SKV, DIM = 2, 2048, 2048, 1024
HEADS, DH = 16, 64
N_CORES = 8
HPC = HEADS // N_CORES  # heads per core = 2
HD = HPC * DH  # 128 head-dim rows per core
TOK = B * SQ  # 4096
KO = DIM // 128  # 8 contraction chunks of 128
SCALE = DH**-0.5

BF16 = mybir.dt.bfloat16
F32 = mybir.dt.float32

PCHUNK = 512  # token chunk in projections (contiguous per-chunk dram layout)
QTILE = 512  # q tile in attention
KTILE = 128  # k tile (scores psum partition dim)
NKT = SKV // KTILE  # 16
NQT = SQ // QTILE  # 4

BF = ml_dtypes.bfloat16
Exp = mybir.ActivationFunctionType.Exp
I16 = mybir.dt.int16
Mult = mybir.AluOpType.mult
Add = mybir.AluOpType.add

# bf16-bitspace Schraudolph exp: bits = rint(s*SCALE*128/ln2 + (127*128 - 5.49))
# bitcast int16->bf16 gives ~exp(s*SCALE) with ~2% rms sawtooth error that is
# common-mode-cancelled by the softmax normalization.  Used on DVE_J steps to
# offload the ACT engine (the attention-stream bottleneck).
SCH_C1 = float(SCALE * 128.0 / np.log(2.0))
SCH_C2 = 16256.0 - 5.49
DVE_J = (3, 8, 10, 12, 14)


def build():
    nc = bacc.Bacc(
        "TRN2", target_bir_lowering=False, debug=False, num_devices=N_CORES
    )

    NCH = TOK // PCHUNK
    xqt_d = nc.dram_tensor("xqt", [NCH, 128, KO, PCHUNK], BF16, kind="ExternalInput")
    xkvt_d = nc.dram_tensor("xkvt", [NCH, 128, KO, PCHUNK], BF16, kind="ExternalInput")
    wq_d = nc.dram_tensor("wq", [DIM, HD], BF16, kind="ExternalInput")
    wk_d = nc.dram_tensor("wk", [DIM, HD], BF16, kind="ExternalInput")
    wv_d = nc.dram_tensor("wv", [DIM, HD], BF16, kind="ExternalInput")
    wout_d = nc.dram_tensor("wout", [HD, DIM], BF16, kind="ExternalInput")
    out_d = nc.dram_tensor("out", [TOK, DIM], BF16, kind="ExternalOutput")

    xqt = xqt_d.ap()
    xkvt = xkvt_d.ap()

    with tile.TileContext(nc) as tc:
        with (
            tc.tile_pool(name="persist", bufs=1) as persist,
            tc.tile_pool(name="xin", bufs=4) as xin,
            tc.tile_pool(name="exps", bufs=10) as exps,
            tc.tile_pool(name="ost", bufs=3) as ost,
            tc.tile_pool(name="spsum", bufs=2, space="PSUM") as spsum,
            tc.tile_pool(name="accp", bufs=2, space="PSUM") as accp,
            tc.tile_pool(name="miscp", bufs=2, space="PSUM") as miscp,
        ):
            # --- weights / constants.  Queue plan for the critical start:
            # sync:   wk, xb-kv0, wq, xb-q0, ...
            # gpsimd: xa-kv0, xa-q0, wv, wout, ...
            # so both queues stream k0/q0 bytes from the first microsecond
            # (per-queue DMA bw is ~85GB/s; the start is transfer-bound). ---
            wk_sb = persist.tile([128, KO, HD], BF16, tag="wk")
            nc.sync.dma_start(wk_sb[:], wk_d.ap().rearrange("(ko p) m -> p ko m", p=128))

            ident = persist.tile([128, DH], BF16, tag="ident")
            make_identity(nc, ident[0:DH, :])
            make_identity(nc, ident[DH : 2 * DH, :])
            ones_f32 = persist.tile([128, DH], F32, tag="ones")
            nc.vector.memset(ones_f32[:], 1.0)
            ones_bf = persist.tile([1, DH], BF16, tag="onesb")
            nc.vector.memset(ones_bf[:], 1.0)

            # PE p-state warmup: ~3us of dummy matmuls during the initial DMA
            # wait so the first real projections run at full clock (the PE
            # ramps 1.2 -> 2.4GHz only after ~3us of continuous execution)
            warm = persist.tile([128, 512], BF16, tag="warm")
            nc.vector.memset(warm[:], 0.0)
            for _ in range(13):
                wps = miscp.tile([128, 512], F32, tag="m", name="warmup")
                nc.tensor.matmul(wps[:], warm[:, 0:128], warm[:], start=True, stop=True)
            # prefetch the exp table set during the head DMAs
            dummy = persist.tile([1, 8], F32, tag="dummy")
            nc.vector.memset(dummy[:], 0.0)
            nc.scalar.activation(dummy[:], dummy[:], Exp)

            qt_sb, kt_sb, vt_sb, vnat, outT, usb = {}, {}, {}, {}, {}, {}
            for b in range(B):
                qt_sb[b] = persist.tile([HD, SQ], BF16, tag=f"qt{b}", name=f"qt{b}")
                kt_sb[b] = persist.tile([HD, SKV], BF16, tag=f"kt{b}", name=f"kt{b}")
                vt_sb[b] = persist.tile([HD, SKV], BF16, tag=f"vt{b}", name=f"vt{b}")
                vnat[b] = persist.tile(
                    [128, HPC, NKT, DH + 1], BF16, tag=f"vn{b}", name=f"vn{b}"
                )
                outT[b] = persist.tile([HD, SQ], BF16, tag=f"ot{b}", name=f"ot{b}")
                # unnormalized outT + denominators, unit index = qt*HPC + h
                usb[b] = persist.tile(
                    [DH + 1, NQT * HPC, QTILE], F32, tag=f"us{b}", name=f"us{b}"
                )
                nc.vector.memset(vnat[b][:, :, :, DH], 1.0)

            def _proj(dst, w_sb, xt, tt):
                for sub in range(PCHUNK // 512):
                    _proj_sub(dst, w_sb, xt, tt, sub)

            KOH = KO // 2

            def load_chunk(x_ap, tok0, tt, engs=None, fine=False):
                """Load a 512-token chunk split by ko across two DMA queues:
                the projection's ko-chain starts as soon as the first piece
                lands.  fine=True uses 4 quarter-tiles (2 per queue) for the
                startup-critical chunks so the first matmuls gate on 256KB
                instead of 512KB."""
                ea, eb = engs or (nc.sync, nc.gpsimd)
                ch = x_ap[(tok0 + tt * PCHUNK) // PCHUNK]
                nt = 4 if fine else 2
                w = KO // nt
                tiles = []
                for i in range(nt):
                    xt = xin.tile(
                        [128, w, PCHUNK], BF16, tag=f"xf{i}" if fine else "ab"[i] + "x"
                    )
                    (ea if i % 2 == 0 else eb).dma_start(
                        xt[:], ch[:, i * w : (i + 1) * w, :]
                    )
                    tiles.append(xt)
                return tiles

            def _proj_sub(dst, w_sb, xt, tt, sub):
                ps = miscp.tile([128, 512], F32, tag="m", name="projp")
                w = KO // len(xt)
                for ko in range(KO):
                    part = xt[ko // w]
                    nc.tensor.matmul(
                        ps[:],
                        w_sb[:, ko, :],
                        part[:, ko % w, sub * 512 : (sub + 1) * 512],
                        start=(ko == 0),
                        stop=(ko == KO - 1),
                    )
                t0 = tt * PCHUNK + sub * 512
                nc.vector.tensor_copy(dst[:, t0 : t0 + 512], ps[:])

            def vnat_group(b, jg):
                """PE-transpose k-tiles 4jg..4jg+3 of vT into natural layout.

                The two heads' transposes alternate partition bands (rows
                0-63 / 64-127) into the two miscp psum bufs, so consecutive
                pairs stream concurrently on the PE."""
                tps = [
                    miscp.tile([128, 4, DH], BF16, tag="m", name="vtp")
                    for _ in range(HPC)
                ]
                for i in range(4):
                    j = jg * 4 + i
                    for h in range(HPC):
                        nc.tensor.transpose(
                            tps[h][:, i, :],
                            vt_sb[b][
                                h * DH : (h + 1) * DH,
                                j * KTILE : (j + 1) * KTILE,
                            ],
                            ident[h * DH : (h + 1) * DH, :],
                        )
                for h in range(HPC):
                    nc.vector.tensor_copy(
                        vnat[b][:, h, jg * 4 : (jg + 1) * 4, 0:DH], tps[h][:]
                    )

            F32R = mybir.dt.float32r

            def norm_flush(b, u0, nu, tail=False):
                """Normalize units u0..u0+nu-1 of usb[b] into outT[b].

                Broadcast-first: the denominator row is spread to 64
                partitions with a K=1 PE outer product, the reciprocal runs
                WIDE on the broadcast (single-partition DVE ops are ~8x
                slower per element), then one DVE multiply.  No DRAM bounce.
                Mid-stream, the row is first cast to bf16 on ACT so the
                broadcast matmul is 1 cycle/row instead of f32's 4; the tail
                path skips the cast for a shorter critical chain.
                """
                if not tail:
                    dbf = ost.tile([1, nu, QTILE], BF16, tag="db", name="dbf")
                    nc.scalar.copy(dbf[:], usb[b][DH : DH + 1, u0 : u0 + nu, :])
                for i in range(nu):
                    g = u0 + i
                    qt, h = divmod(g, HPC)
                    bcd = miscp.tile([DH, QTILE], F32, tag="m", name="bcd")
                    if tail:
                        nc.tensor.matmul(
                            bcd[:],
                            ones_f32[DH : DH + 1, :],
                            usb[b][DH : DH + 1, g, :],
                            start=True,
                            stop=True,
                        )
                    else:
                        nc.tensor.matmul(
                            bcd[:],
                            ones_bf[:],
                            dbf[0:1, i, :],
                            start=True,
                            stop=True,
                        )
                    rbc = ost.tile([DH, QTILE], F32, tag="rb", name="rbc")
                    nc.vector.reciprocal_approx_fast(rbc[:], bcd[:])
                    nc.vector.tensor_mul(
                        outT[b][h * DH : (h + 1) * DH, qt * QTILE : (qt + 1) * QTILE],
                        usb[b][0:DH, g, :],
                        rbc[:],
                    )

            LOOKAHEAD = 6

            def attention(b, hooks, pre=None):
                """Flat software-pipelined attention over all (qt, j) steps.

                Scores for step t+2 are emitted before PV of step t, so the
                PE always has score matmuls queued ahead of the exp/PV chain
                and q-tile boundaries pipeline seamlessly.  hooks is a dict
                keyed (qt, j) of emission callables fired right after that
                step's PV matmuls; pre is keyed by flat step t and fires
                BEFORE that step's scores (for work the scores depend on,
                e.g. the batch's own later qkv chunks).
                """
                NT = NQT * NKT
                sps, accs = {}, {}
                pre = pre or {}

                def emit_scores(t):
                    qt, j = divmod(t, NKT)
                    q_sl = slice(qt * QTILE, (qt + 1) * QTILE)
                    k_sl = slice(j * KTILE, (j + 1) * KTILE)
                    sp = spsum.tile([128, HPC, QTILE], F32, tag="s", name="sp")
                    sps[t] = sp
                    for h in range(HPC):
                        h_sl = slice(h * DH, (h + 1) * DH)
                        nc.tensor.matmul(
                            sp[:, h, :],
                            kt_sb[b][h_sl, k_sl],
                            qt_sb[b][h_sl, q_sl],
                            start=True,
                            stop=True,
                        )

                def emit_tail(t):
                    qt, j = divmod(t, NKT)
                    sp = sps.pop(t)
                    if j in DVE_J:
                        # fast-exp on DVE (bf16 bitspace), offloading ACT
                        exi = exps.tile([128, HPC, QTILE], I16, tag="ei", name="exi")
                        nc.vector.tensor_scalar(
                            out=exi[:],
                            in0=sp[:],
                            scalar1=SCH_C1,
                            scalar2=SCH_C2,
                            op0=Mult,
                            op1=Add,
                        )
                        ex = exi[:].bitcast(BF16)
                    else:
                        exf = exps.tile([128, HPC, QTILE], BF16, tag="e", name="ex")
                        nc.scalar.activation(exf[:], sp[:], Exp, scale=SCALE)
                        ex = exf[:]
                    if j == 0:
                        accs[qt] = [
                            accp.tile([128, QTILE], F32, tag="acc", name="acc")
                            for _ in range(HPC)
                        ]
                    for h in range(HPC):
                        nc.tensor.matmul(
                            accs[qt][h][0 : DH + 1, :],
                            vnat[b][:, h, j, :],
                            ex[:, h, :],
                            start=(j == 0),
                            stop=(j == NKT - 1),
                        )
                    if j == NKT - 1:
                        for h in range(HPC):
                            # free the PSUM accumulator; normalization comes
                            # later in norm_flush
                            nc.vector.tensor_copy(
                                usb[b][:, qt * HPC + h, :],
                                accs[qt][h][0 : DH + 1, :],
                            )
                        del accs[qt]
                    for fn in hooks.get((qt, j), []):
                        fn()

                for t in range(NT + LOOKAHEAD):
                    if t < NT:
                        for fn in pre.get(t, []):
                            fn()
                        emit_scores(t)
                    if t >= LOOKAHEAD:
                        emit_tail(t - LOOKAHEAD)

            def outproj(b, tt0, tt1, split_copy=False, pools=None):
                for tt in range(tt0, tt1):
                    t_sl = slice(tt * 128, (tt + 1) * 128)
                    ob = ost.tile([128, 2, 512], BF16, tag="o")
                    for nt in range(DIM // 512):
                        if pools is None:
                            ps = miscp.tile([128, 512], F32, tag="m", name="projo")
                        else:
                            # tail: spread across free PSUM pools so the
                            # matmuls aren't gated by copy-recycle latency
                            pool, ptag = pools[(tt * 2 + nt) % len(pools)]
                            ps = pool.tile([128, 512], F32, tag=ptag, name="projo")
                        nc.tensor.matmul(
                            ps[:],
                            outT[b][:, t_sl],
                            wout_sb[:, nt * 512 : (nt + 1) * 512],
                            start=True,
                            stop=True,
                        )
                        if split_copy and nt % 2 == 0:
                            nc.scalar.copy(ob[:, nt, :], ps[:])
                        else:
                            nc.vector.tensor_copy(ob[:, nt, :], ps[:])
                        if pools is not None:
                            # tail: ship each half as soon as its copy lands,
                            # alternating queues, so the final DMA drain
                            # overlaps the remaining matmuls
                            eng = nc.gpsimd if (tt * 2 + nt) % 2 else nc.sync
                            eng.dma_start(
                                out_d.ap()[
                                    b * SQ + tt * 128 : b * SQ + (tt + 1) * 128,
                                    nt * 512 : (nt + 1) * 512,
                                ],
                                ob[:, nt, :],
                            )
                    if pools is None:
                        (nc.gpsimd if tt % 2 else nc.sync).dma_start(
                            out_d.ap()[
                                b * SQ + tt * 128 : b * SQ + (tt + 1) * 128, :
                            ].rearrange("t (n c) -> t n c", n=2),
                            ob[:],
                        )

            def qkv_pieces(b):
                """Projection emission steps, 512-token chunks.  K chunks
                first (scores consume them progressively); each x_kv chunk
                is loaded once for both K and V; Q tiles beyond the first
                q-tile come last."""
                xts = {}

                def kv_load_k(tt):
                    xts[tt] = load_chunk(xkvt, b * SKV, tt)
                    _proj(kt_sb[b], wk_sb, xts[tt], tt)

                def v_part(tt):
                    _proj(vt_sb[b], wv_sb, xts.pop(tt), tt)
                    vnat_group(b, tt)

                yield lambda: kv_load_k(0)
                yield lambda: proj_chunk(qt_sb[b], wq_sb, xqt, b * SQ, 0)
                yield lambda: v_part(0)
                for tt in range(1, SQ // PCHUNK):
                    yield lambda tt=tt: kv_load_k(tt)
                    yield lambda tt=tt: v_part(tt)
                for tt in range(1, SQ // PCHUNK):
                    yield lambda tt=tt: proj_chunk(qt_sb[b], wq_sb, xqt, b * SQ, tt)

            def proj_chunk(dst, w_sb, x_ap, tok0, tt):
                _proj(dst, w_sb, load_chunk(x_ap, tok0, tt), tt)

            # --- emission schedule: batch 0 starts attention right after its
            # first k/q chunks; the rest of its own qkv work is interleaved
            # into the qt=0 window via pre-hooks (scores for chunk c's
            # k-tiles must be emitted after chunk c's projection to keep the
            # in-order PE queue deadlock-free), so the startup is gated by
            # ~2MB of DMA instead of 8MB. ---
            kvx = {0: load_chunk(xkvt, 0, 0, engs=(nc.gpsimd, nc.sync), fine=True)}
            wq_sb = persist.tile([128, KO, HD], BF16, tag="wq")
            nc.sync.dma_start(wq_sb[:], wq_d.ap().rearrange("(ko p) m -> p ko m", p=128))
            # wv must land before v0-proj (~12us in): gpsimd right after the
            # kv0 half.  wout isn't needed until ~40us: scalar queue (its
            # ~5.6us per-descriptor issue cost rides the idle ACT engine and
            # a third hw queue, keeping sync/gpsimd clear for the x chunks)
            wv_sb = persist.tile([128, KO, HD], BF16, tag="wv")
            nc.gpsimd.dma_start(wv_sb[:], wv_d.ap().rearrange("(ko p) m -> p ko m", p=128))
            q0x = load_chunk(xqt, 0, 0, engs=(nc.gpsimd, nc.sync), fine=True)
            wout_sb = persist.tile([HD, DIM], BF16, tag="wout")
            nc.scalar.dma_start(wout_sb[:], wout_d.ap())
            _proj(kt_sb[0], wk_sb, kvx[0], 0)
            _proj(qt_sb[0], wq_sb, q0x, 0)

            qlx = {}

            def kv_load0(tt):
                return lambda: kvx.__setitem__(tt, load_chunk(xkvt, 0, tt))

            def q_load0(tt):
                return lambda: qlx.__setitem__(tt, load_chunk(xqt, 0, tt))

            def k_proj0(tt):
                return lambda: _proj(kt_sb[0], wk_sb, kvx[tt], tt)

            def v_proj0(tt):
                def go():
                    _proj(vt_sb[0], wv_sb, kvx.pop(tt), tt)
                    vnat_group(0, tt)

                return go

            def q_proj0(tt):
                return lambda: _proj(qt_sb[0], wq_sb, qlx.pop(tt), tt)

            pre0 = {
                0: [kv_load0(1)],
                1: [v_proj0(0)],
                2: [kv_load0(2), k_proj0(1)],
                3: [kv_load0(3)],
                4: [v_proj0(1)],
                6: [k_proj0(2)],
                8: [q_load0(1), v_proj0(2)],
                10: [q_load0(2), k_proj0(3)],
                12: [q_load0(3), v_proj0(3)],
                16: [q_proj0(1)],
                32: [q_proj0(2)],
                48: [q_proj0(3)],
            }

            nxt = qkv_pieces(1)

            def emit_next():
                p = next(nxt, None)
                if p is not None:
                    p()

            def emit_n(n):
                def go():
                    for _ in range(n):
                        emit_next()

                return go

            def flush_op(b, qt):
                def go():
                    norm_flush(b, qt * HPC, HPC)
                    outproj(b, qt * 4, qt * 4 + 4, split_copy=True)

                return go

            hooks0 = {
                (1, 1): [flush_op(0, 0)],
                (1, 15): [emit_n(4)],
                (2, 1): [flush_op(0, 1)],
                (2, 15): [emit_n(4)],
                (3, 1): [flush_op(0, 2)],
                (3, 15): [emit_n(4)],
            }
            attention(0, hooks0, pre0)

            hooks1 = {
                (0, 0): [lambda: norm_flush(0, 3 * HPC, HPC)],
                (0, 3): [emit_next],
                (0, 8): [lambda: outproj(0, 12, 16, split_copy=True), emit_next],
                (0, 13): [emit_next],
                (1, 1): [flush_op(1, 0)],
                (2, 1): [flush_op(1, 1)],
                (3, 1): [flush_op(1, 2)],
            }
            attention(1, hooks1)
            # tail: per-head final norms, then the last outproj spread over
            # 4 psum banks (miscp+accp) so its matmuls pipeline densely
            norm_flush(1, 3 * HPC, 1, tail=True)
            norm_flush(1, 3 * HPC + 1, 1, tail=True)
            outproj(1, 12, 16, split_copy=True, pools=[(miscp, "m"), (accp, "acc")])

    nc.compile()
    return nc


def make_in_maps(x_q, x_kv, W_qkv, W_out):
    x_q = np.asarray(x_q, dtype=np.float32)
    x_kv = np.asarray(x_kv, dtype=np.float32)
    W_qkv = np.asarray(W_qkv, dtype=np.float32)
    W_out = np.asarray(W_out, dtype=np.float32)

    def chunk_tile(x):
        # [TOK, DIM] -> [n_chunks, 128, KO, PCHUNK] with D = ko*128 + p
        xt = x.reshape(TOK, DIM).T.reshape(KO, 128, TOK // PCHUNK, PCHUNK)
        return np.ascontiguousarray(xt.transpose(2, 1, 0, 3)).astype(BF)

    xqt = chunk_tile(x_q)
    xkvt = chunk_tile(x_kv)

    in_maps = []
    for c in range(N_CORES):
        cs = slice(c * HD, (c + 1) * HD)
        in_maps.append(
            {
                "xqt": xqt,
                "xkvt": xkvt,
                "wq": np.ascontiguousarray(W_qkv[:, cs]).astype(BF),
                "wk": np.ascontiguousarray(W_qkv[:, 1024:][:, cs]).astype(BF),
                "wv": np.ascontiguousarray(W_qkv[:, 2048:][:, cs]).astype(BF),
                "wout": np.ascontiguousarray(W_out[cs, :]).astype(BF),
            }
        )
    return in_maps


def combine(partials, b_out):
    """Sum the 8 per-core partial projections and add the bias."""
    acc = np.zeros((TOK, DIM), dtype=np.float32)
    for p in partials:
        acc += np.asarray(p, dtype=np.float32)
    acc += np.asarray(b_out, dtype=np.float32)
    return acc.reshape(B, SQ, DIM)


_STATE = {}


def _get_nc():
    if "nc" not in _STATE:
        _STATE["nc"] = build()
    return _STATE["nc"]


def run(x_q, x_kv, W_qkv, W_out, b_out, trace=False):
    nc = _get_nc()
    in_maps = make_in_maps(x_q, x_kv, W_qkv, W_out)
    res = run_bass_kernel_spmd(nc, in_maps, list(range(N_CORES)), trace=trace)
    out = combine([r["out"] for r in res.results], b_out)
    return out, res


def kernel(x_q, x_kv, W_qkv, W_out, b_out):
    out, _ = run(x_q, x_kv, W_qkv, W_out, b_out, trace=False)
    return out



# revision 51
# speedup vs baseline: 1.1839x; 1.1839x over previous
"""Multi-head attention (b=2, sq=skv=2048, dim=1024, 16 heads x 64) on 8 TRN2
NeuronCores.

Sharding: 2 heads per core (head-parallel across batch*heads), with the
matching tensor-parallel column slice of W_qkv and row slice of W_out.  Each
core computes a partial output projection over its 128 head-dims; the
all-reduce of the 8 partials (+ bias) happens on the host during unshard.

Per-core kernel (bf16 compute, fp32 PSUM accumulation):
  phase 1: qT/kT/vT = W.T @ x.T   ([128 = 2 heads x 64 dims, tokens]); v is
           additionally PE-transposed to natural [token, dim] layout with a
           ones column appended (denominator trick).
  phase 2: per (batch, q-tile, k-tile): scoresT for both heads ([k-tokens, q])
           in one 2-bank PSUM group; one exp ACTIVATE over the group (scale
           1/8 fused, no max subtraction -- scores range +-10); PV matmuls
           accumulate [v | 1].T @ expT over the 16 k-tiles giving unnormalized
           outT plus the softmax denominator in row 64.  The accumulator is
           copied to SBUF immediately (releasing PSUM); normalization
           (reciprocal + PE outer-product broadcast + multiply) happens off
           the critical path.
  phase 3: partial out = outT.T @ W_out_rows -> bf16 [tokens, 1024].

Emission is orchestrated so the dependency-driven Tile scheduler always has
filler PE work (batch-1 projections, out-projection quarters) inside the
ACT(exp)-bound attention stream, keeping the PE HAM-warm.
"""

import os
import sys

for _p in ("/opt/trn_rl_repo", "/root/.axon_site/_ro/trn_rl_repo"):
    if os.path.isdir(_p) and _p not in sys.path:
        sys.path.append(_p)

import ml_dtypes
import numpy as np

import concourse.bass as bass  # noqa: F401
import concourse.tile as tile
from concourse import bacc, mybir
from concourse.bass_utils import run_bass_kernel_spmd
from concourse.masks import make_identity

B, SQ, SKV, DIM = 2, 2048, 2048, 1024
HEADS, DH = 16, 64
N_CORES = 8
HPC = HEADS // N_CORES  # heads per core = 2
HD = HPC * DH  # 128 head-dim rows per core
TOK = B * SQ  # 4096
KO = DIM // 128  # 8 contraction chunks of 128
SCALE = DH**-0.5

BF16 = mybir.dt.bfloat16
F32 = mybir.dt.float32

PCHUNK = 512  # token chunk in projections (contiguous per-chunk dram layout)
QTILE = 512  # q tile in attention
KTILE = 128  # k tile (scores psum partition dim)
NKT = SKV // KTILE  # 16
NQT = SQ // QTILE  # 4

BF = ml_dtypes.bfloat16
Exp = mybir.ActivationFunctionType.Exp
I16 = mybir.dt.int16
Mult = mybir.AluOpType.mult
Add = mybir.AluOpType.add

# bf16-bitspace Schraudolph exp: bits = rint(s*SCALE*128/ln2 + (127*128 - 5.49))
# bitcast int16->bf16 gives ~exp(s*SCALE) with ~2% rms sawtooth error that is
# common-mode-cancelled by the softmax normalization.  Used on DVE_J steps to
# offload the ACT engine (the attention-stream bottleneck).
SCH_C1 = float(SCALE * 128.0 / np.log(2.0))
SCH_C2 = 16256.0 - 5.49
DVE_J = (3, 8, 10, 12, 14)


def build():
    nc = bacc.Bacc(
        "TRN2", target_bir_lowering=False, debug=False, num_devices=N_CORES
    )

    NCH = TOK // PCHUNK
    xqt_d = nc.dram_tensor("xqt", [NCH, 128, KO, PCHUNK], BF16, kind="ExternalInput")
    xkvt_d = nc.dram_tensor("xkvt", [NCH, 128, KO, PCHUNK], BF16, kind="ExternalInput")
    wq_d = nc.dram_tensor("wq", [DIM, HD], BF16, kind="ExternalInput")
    wk_d = nc.dram_tensor("wk", [DIM, HD], BF16, kind="ExternalInput")
    wv_d = nc.dram_tensor("wv", [DIM, HD], BF16, kind="ExternalInput")
    wout_d = nc.dram_tensor("wout", [HD, DIM], BF16, kind="ExternalInput")
    out_d = nc.dram_tensor("out", [TOK, DIM], BF16, kind="ExternalOutput")

    xqt = xqt_d.ap()
    xkvt = xkvt_d.ap()

    with tile.TileContext(nc) as tc:
        with (
            tc.tile_pool(name="persist", bufs=1) as persist,
            tc.tile_pool(name="xin", bufs=4) as xin,
            tc.tile_pool(name="exps", bufs=10) as exps,
            tc.tile_pool(name="ost", bufs=3) as ost,
            tc.tile_pool(name="spsum", bufs=2, space="PSUM") as spsum,
            tc.tile_pool(name="accp", bufs=2, space="PSUM") as accp,
            tc.tile_pool(name="miscp", bufs=2, space="PSUM") as miscp,
        ):
            # --- weights / constants.  Queue plan for the critical start:
            # sync:   wk, xb-kv0, wq, xb-q0, ...
            # gpsimd: xa-kv0, xa-q0, wv, wout, ...
            # so both queues stream k0/q0 bytes from the first microsecond
            # (per-queue DMA bw is ~85GB/s; the start is transfer-bound). ---
            wk_sb = persist.tile([128, KO, HD], BF16, tag="wk")
            nc.sync.dma_start(wk_sb[:], wk_d.ap().rearrange("(ko p) m -> p ko m", p=128))

            ident = persist.tile([128, DH], BF16, tag="ident")
            make_identity(nc, ident[0:DH, :])
            make_identity(nc, ident[DH : 2 * DH, :])
            ones_f32 = persist.tile([128, DH], F32, tag="ones")
            nc.vector.memset(ones_f32[:], 1.0)
            ones_bf = persist.tile([1, DH], BF16, tag="onesb")
            nc.vector.memset(ones_bf[:], 1.0)

            # PE p-state warmup: ~3us of dummy matmuls during the initial DMA
            # wait so the first real projections run at full clock (the PE
            # ramps 1.2 -> 2.4GHz only after ~3us of continuous execution)
            warm = persist.tile([128, 512], BF16, tag="warm")
            nc.vector.memset(warm[:], 0.0)
            for _ in range(8):
                wps = miscp.tile([128, 512], F32, tag="m", name="warmup")
                nc.tensor.matmul(wps[:], warm[:, 0:128], warm[:], start=True, stop=True)
            # prefetch the exp table set during the head DMAs
            dummy = persist.tile([1, 8], F32, tag="dummy")
            nc.vector.memset(dummy[:], 0.0)
            nc.scalar.activation(dummy[:], dummy[:], Exp)

            qt_sb, kt_sb, vt_sb, vnat, outT, usb = {}, {}, {}, {}, {}, {}
            for b in range(B):
                qt_sb[b] = persist.tile([HD, SQ], BF16, tag=f"qt{b}", name=f"qt{b}")
                kt_sb[b] = persist.tile([HD, SKV], BF16, tag=f"kt{b}", name=f"kt{b}")
                vt_sb[b] = persist.tile([HD, SKV], BF16, tag=f"vt{b}", name=f"vt{b}")
                vnat[b] = persist.tile(
                    [128, HPC, NKT, DH + 1], BF16, tag=f"vn{b}", name=f"vn{b}"
                )
                outT[b] = persist.tile([HD, SQ], BF16, tag=f"ot{b}", name=f"ot{b}")
                # unnormalized outT + denominators, unit index = qt*HPC + h
                usb[b] = persist.tile(
                    [DH + 1, NQT * HPC, QTILE], F32, tag=f"us{b}", name=f"us{b}"
                )
                nc.vector.memset(vnat[b][:, :, :, DH], 1.0)

            def _proj(dst, w_sb, xt, tt):
                for sub in range(PCHUNK // 512):
                    _proj_sub(dst, w_sb, xt, tt, sub)

            KOH = KO // 2

            def load_chunk(x_ap, tok0, tt, engs=None, fine=False):
                """Load a 512-token chunk split by ko across two DMA queues:
                the projection's ko-chain starts as soon as the first piece
                lands.  fine=True uses 4 quarter-tiles (2 per queue) for the
                startup-critical chunks so the first matmuls gate on 256KB
                instead of 512KB."""
                ea, eb = engs or (nc.sync, nc.gpsimd)
                ch = x_ap[(tok0 + tt * PCHUNK) // PCHUNK]
                nt = 4 if fine else 2
                w = KO // nt
                tiles = []
                for i in range(nt):
                    xt = xin.tile(
                        [128, w, PCHUNK], BF16, tag=f"xf{i}" if fine else "ab"[i] + "x"
                    )
                    (ea if i % 2 == 0 else eb).dma_start(
                        xt[:], ch[:, i * w : (i + 1) * w, :]
                    )
                    tiles.append(xt)
                return tiles

            def _proj_sub(dst, w_sb, xt, tt, sub):
                ps = miscp.tile([128, 512], F32, tag="m", name="projp")
                w = KO // len(xt)
                for ko in range(KO):
                    part = xt[ko // w]
                    nc.tensor.matmul(
                        ps[:],
                        w_sb[:, ko, :],
                        part[:, ko % w, sub * 512 : (sub + 1) * 512],
                        start=(ko == 0),
                        stop=(ko == KO - 1),
                    )
                t0 = tt * PCHUNK + sub * 512
                nc.vector.tensor_copy(dst[:, t0 : t0 + 512], ps[:])

            def vnat_group(b, jg):
                """PE-transpose k-tiles 4jg..4jg+3 of vT into natural layout.

                The two heads' transposes alternate partition bands (rows
                0-63 / 64-127) into the two miscp psum bufs, so consecutive
                pairs stream concurrently on the PE."""
                tps = [
                    miscp.tile([128, 4, DH], BF16, tag="m", name="vtp")
                    for _ in range(HPC)
                ]
                for i in range(4):
                    j = jg * 4 + i
                    for h in range(HPC):
                        nc.tensor.transpose(
                            tps[h][:, i, :],
                            vt_sb[b][
                                h * DH : (h + 1) * DH,
                                j * KTILE : (j + 1) * KTILE,
                            ],
                            ident[h * DH : (h + 1) * DH, :],
                        )
                for h in range(HPC):
                    nc.vector.tensor_copy(
                        vnat[b][:, h, jg * 4 : (jg + 1) * 4, 0:DH], tps[h][:]
                    )

            F32R = mybir.dt.float32r

            def norm_flush(b, u0, nu, tail=False):
                """Normalize units u0..u0+nu-1 of usb[b] into outT[b].

                Broadcast-first: the denominator row is spread to 64
                partitions with a K=1 PE outer product, the reciprocal runs
                WIDE on the broadcast (single-partition DVE ops are ~8x
                slower per element), then one DVE multiply.  No DRAM bounce.
                Mid-stream, the row is first cast to bf16 on ACT so the
                broadcast matmul is 1 cycle/row instead of f32's 4; the tail
                path skips the cast for a shorter critical chain.
                """
                if not tail:
                    dbf = ost.tile([1, nu, QTILE], BF16, tag="db", name="dbf")
                    nc.scalar.copy(dbf[:], usb[b][DH : DH + 1, u0 : u0 + nu, :])
                for i in range(nu):
                    g = u0 + i
                    qt, h = divmod(g, HPC)
                    bcd = miscp.tile([DH, QTILE], F32, tag="m", name="bcd")
                    if tail:
                        nc.tensor.matmul(
                            bcd[:],
                            ones_f32[DH : DH + 1, :],
                            usb[b][DH : DH + 1, g, :],
                            start=True,
                            stop=True,
                        )
                    else:
                        nc.tensor.matmul(
                            bcd[:],
                            ones_bf[:],
                            dbf[0:1, i, :],
                            start=True,
                            stop=True,
                        )
                    rbc = ost.tile([DH, QTILE], F32, tag="rb", name="rbc")
                    nc.vector.reciprocal_approx_fast(rbc[:], bcd[:])
                    nc.vector.tensor_mul(
                        outT[b][h * DH : (h + 1) * DH, qt * QTILE : (qt + 1) * QTILE],
                        usb[b][0:DH, g, :],
                        rbc[:],
                    )

            LOOKAHEAD = 6

            def attention(b, hooks, pre=None):
                """Flat software-pipelined attention over all (qt, j) steps.

                Scores for step t+2 are emitted before PV of step t, so the
                PE always has score matmuls queued ahead of the exp/PV chain
                and q-tile boundaries pipeline seamlessly.  hooks is a dict
                keyed (qt, j) of emission callables fired right after that
                step's PV matmuls; pre is keyed by flat step t and fires
                BEFORE that step's scores (for work the scores depend on,
                e.g. the batch's own later qkv chunks).
                """
                NT = NQT * NKT
                sps, accs = {}, {}
                pre = pre or {}

                def emit_scores(t):
                    qt, j = divmod(t, NKT)
                    q_sl = slice(qt * QTILE, (qt + 1) * QTILE)
                    k_sl = slice(j * KTILE, (j + 1) * KTILE)
                    sp = spsum.tile([128, HPC, QTILE], F32, tag="s", name="sp")
                    sps[t] = sp
                    for h in range(HPC):
                        h_sl = slice(h * DH, (h + 1) * DH)
                        nc.tensor.matmul(
                            sp[:, h, :],
                            kt_sb[b][h_sl, k_sl],
                            qt_sb[b][h_sl, q_sl],
                            start=True,
                            stop=True,
                        )

                def emit_tail(t):
                    qt, j = divmod(t, NKT)
                    sp = sps.pop(t)
                    if j in DVE_J:
                        # fast-exp on DVE (bf16 bitspace), offloading ACT
                        exi = exps.tile([128, HPC, QTILE], I16, tag="ei", name="exi")
                        nc.vector.tensor_scalar(
                            out=exi[:],
                            in0=sp[:],
                            scalar1=SCH_C1,
                            scalar2=SCH_C2,
                            op0=Mult,
                            op1=Add,
                        )
                        ex = exi[:].bitcast(BF16)
                    else:
                        exf = exps.tile([128, HPC, QTILE], BF16, tag="e", name="ex")
                        nc.scalar.activation(exf[:], sp[:], Exp, scale=SCALE)
                        ex = exf[:]
                    if j == 0:
                        accs[qt] = [
                            accp.tile([128, QTILE], F32, tag="acc", name="acc")
                            for _ in range(HPC)
                        ]
                    for h in range(HPC):
                        nc.tensor.matmul(
                            accs[qt][h][0 : DH + 1, :],
                            vnat[b][:, h, j, :],
                            ex[:, h, :],
                            start=(j == 0),
                            stop=(j == NKT - 1),
                        )
                    if j == NKT - 1:
                        for h in range(HPC):
                            # free the PSUM accumulator; normalization comes
                            # later in norm_flush
                            nc.vector.tensor_copy(
                                usb[b][:, qt * HPC + h, :],
                                accs[qt][h][0 : DH + 1, :],
                            )
                        del accs[qt]
                    for fn in hooks.get((qt, j), []):
                        fn()

                for t in range(NT + LOOKAHEAD):
                    if t < NT:
                        for fn in pre.get(t, []):
                            fn()
                        emit_scores(t)
                    if t >= LOOKAHEAD:
                        emit_tail(t - LOOKAHEAD)

            def outproj(b, tt0, tt1, split_copy=False, pools=None):
                for tt in range(tt0, tt1):
                    t_sl = slice(tt * 128, (tt + 1) * 128)
                    ob = ost.tile([128, 2, 512], BF16, tag="o")
                    for nt in range(DIM // 512):
                        if pools is None:
                            ps = miscp.tile([128, 512], F32, tag="m", name="projo")
                        else:
                            # tail: spread across free PSUM pools so the
                            # matmuls aren't gated by copy-recycle latency
                            pool, ptag = pools[(tt * 2 + nt) % len(pools)]
                            ps = pool.tile([128, 512], F32, tag=ptag, name="projo")
                        nc.tensor.matmul(
                            ps[:],
                            outT[b][:, t_sl],
                            wout_sb[:, nt * 512 : (nt + 1) * 512],
                            start=True,
                            stop=True,
                        )
                        if split_copy and nt % 2 == 0:
                            nc.scalar.copy(ob[:, nt, :], ps[:])
                        else:
                            nc.vector.tensor_copy(ob[:, nt, :], ps[:])
                        if pools is not None:
                            # tail: ship each half as soon as its copy lands,
                            # alternating queues, so the final DMA drain
                            # overlaps the remaining matmuls
                            eng = nc.gpsimd if (tt * 2 + nt) % 2 else nc.sync
                            eng.dma_start(
                                out_d.ap()[
                                    b * SQ + tt * 128 : b * SQ + (tt + 1) * 128,
                                    nt * 512 : (nt + 1) * 512,
                                ],
                                ob[:, nt, :],
                            )
                    if pools is None:
                        (nc.gpsimd if tt % 2 else nc.sync).dma_start(
                            out_d.ap()[
                                b * SQ + tt * 128 : b * SQ + (tt + 1) * 128, :
                            ].rearrange("t (n c) -> t n c", n=2),
                            ob[:],
                        )

            def qkv_pieces(b):
                """Projection emission steps, 512-token chunks.  K chunks
                first (scores consume them progressively); each x_kv chunk
                is loaded once for both K and V; Q tiles beyond the first
                q-tile come last."""
                xts = {}

                def kv_load_k(tt):
                    xts[tt] = load_chunk(xkvt, b * SKV, tt)
                    _proj(kt_sb[b], wk_sb, xts[tt], tt)

                def v_part(tt):
                    _proj(vt_sb[b], wv_sb, xts.pop(tt), tt)
                    vnat_group(b, tt)

                yield lambda: kv_load_k(0)
                yield lambda: proj_chunk(qt_sb[b], wq_sb, xqt, b * SQ, 0)
                yield lambda: v_part(0)
                for tt in range(1, SQ // PCHUNK):
                    yield lambda tt=tt: kv_load_k(tt)
                    yield lambda tt=tt: v_part(tt)
                for tt in range(1, SQ // PCHUNK):
                    yield lambda tt=tt: proj_chunk(qt_sb[b], wq_sb, xqt, b * SQ, tt)

            def proj_chunk(dst, w_sb, x_ap, tok0, tt):
                _proj(dst, w_sb, load_chunk(x_ap, tok0, tt), tt)

            # --- emission schedule: batch 0 starts attention right after its
            # first k/q chunks; the rest of its own qkv work is interleaved
            # into the qt=0 window via pre-hooks (scores for chunk c's
            # k-tiles must be emitted after chunk c's projection to keep the
            # in-order PE queue deadlock-free), so the startup is gated by
            # ~2MB of DMA instead of 8MB. ---
            kvx = {0: load_chunk(xkvt, 0, 0, engs=(nc.gpsimd, nc.sync), fine=True)}
            wq_sb = persist.tile([128, KO, HD], BF16, tag="wq")
            nc.sync.dma_start(wq_sb[:], wq_d.ap().rearrange("(ko p) m -> p ko m", p=128))
            # wv must land before v0-proj (~12us in): gpsimd right after the
            # kv0 half.  wout isn't needed until ~40us: scalar queue (its
            # ~5.6us per-descriptor issue cost rides the idle ACT engine and
            # a third hw queue, keeping sync/gpsimd clear for the x chunks)
            wv_sb = persist.tile([128, KO, HD], BF16, tag="wv")
            nc.gpsimd.dma_start(wv_sb[:], wv_d.ap().rearrange("(ko p) m -> p ko m", p=128))
            q0x = load_chunk(xqt, 0, 0, engs=(nc.gpsimd, nc.sync), fine=True)
            wout_sb = persist.tile([HD, DIM], BF16, tag="wout")
            nc.scalar.dma_start(wout_sb[:], wout_d.ap())
            _proj(kt_sb[0], wk_sb, kvx[0], 0)
            _proj(qt_sb[0], wq_sb, q0x, 0)

            qlx = {}

            def kv_load0(tt):
                return lambda: kvx.__setitem__(tt, load_chunk(xkvt, 0, tt))

            def q_load0(tt):
                return lambda: qlx.__setitem__(tt, load_chunk(xqt, 0, tt))

            def k_proj0(tt):
                return lambda: _proj(kt_sb[0], wk_sb, kvx[tt], tt)

            def v_proj0(tt):
                def go():
                    _proj(vt_sb[0], wv_sb, kvx.pop(tt), tt)
                    vnat_group(0, tt)

                return go

            def q_proj0(tt):
                return lambda: _proj(qt_sb[0], wq_sb, qlx.pop(tt), tt)

            pre0 = {
                0: [kv_load0(1)],
                1: [v_proj0(0)],
                2: [kv_load0(2), k_proj0(1)],
                3: [kv_load0(3)],
                4: [v_proj0(1)],
                6: [k_proj0(2)],
                8: [q_load0(1), v_proj0(2)],
                10: [q_load0(2), k_proj0(3)],
                12: [q_load0(3), v_proj0(3)],
                16: [q_proj0(1)],
                32: [q_proj0(2)],
                48: [q_proj0(3)],
            }

            nxt = qkv_pieces(1)

            def emit_next():
                p = next(nxt, None)
                if p is not None:
                    p()

            def emit_n(n):
                def go():
                    for _ in range(n):
                        emit_next()

                return go

            def flush_op(b, qt):
                def go():
                    norm_flush(b, qt * HPC, HPC)
                    outproj(b, qt * 4, qt * 4 + 4, split_copy=True)

                return go

            hooks0 = {
                (1, 1): [flush_op(0, 0)],
                (1, 15): [emit_n(4)],
                (2, 1): [flush_op(0, 1)],
                (2, 15): [emit_n(4)],
                (3, 1): [flush_op(0, 2)],
                (3, 15): [emit_n(4)],
            }
            attention(0, hooks0, pre0)

            hooks1 = {
                (0, 0): [lambda: norm_flush(0, 3 * HPC, HPC)],
                (0, 3): [emit_next],
                (0, 8): [lambda: outproj(0, 12, 16, split_copy=True), emit_next],
                (0, 13): [emit_next],
                (1, 1): [flush_op(1, 0)],
                (2, 1): [flush_op(1, 1)],
                (3, 1): [flush_op(1, 2)],
            }
            attention(1, hooks1)
            # tail: per-head final norms, then the last outproj spread over
            # 4 psum banks (miscp+accp) so its matmuls pipeline densely
            norm_flush(1, 3 * HPC, 1, tail=True)
            norm_flush(1, 3 * HPC + 1, 1, tail=True)
            outproj(1, 12, 16, split_copy=True, pools=[(miscp, "m"), (accp, "acc")])

    nc.compile()
    return nc


def make_in_maps(x_q, x_kv, W_qkv, W_out):
    x_q = np.asarray(x_q, dtype=np.float32)
    x_kv = np.asarray(x_kv, dtype=np.float32)
    W_qkv = np.asarray(W_qkv, dtype=np.float32)
    W_out = np.asarray(W_out, dtype=np.float32)

    def chunk_tile(x):
        # [TOK, DIM] -> [n_chunks, 128, KO, PCHUNK] with D = ko*128 + p
        xt = x.reshape(TOK, DIM).T.reshape(KO, 128, TOK // PCHUNK, PCHUNK)
        return np.ascontiguousarray(xt.transpose(2, 1, 0, 3)).astype(BF)

    xqt = chunk_tile(x_q)
    xkvt = chunk_tile(x_kv)

    in_maps = []
    for c in range(N_CORES):
        cs = slice(c * HD, (c + 1) * HD)
        in_maps.append(
            {
                "xqt": xqt,
                "xkvt": xkvt,
                "wq": np.ascontiguousarray(W_qkv[:, cs]).astype(BF),
                "wk": np.ascontiguousarray(W_qkv[:, 1024:][:, cs]).astype(BF),
                "wv": np.ascontiguousarray(W_qkv[:, 2048:][:, cs]).astype(BF),
                "wout": np.ascontiguousarray(W_out[cs, :]).astype(BF),
            }
        )
    return in_maps


def combine(partials, b_out):
    """Sum the 8 per-core partial projections and add the bias."""
    acc = np.zeros((TOK, DIM), dtype=np.float32)
    for p in partials:
        acc += np.asarray(p, dtype=np.float32)
    acc += np.asarray(b_out, dtype=np.float32)
    return acc.reshape(B, SQ, DIM)


_STATE = {}


def _get_nc():
    if "nc" not in _STATE:
        _STATE["nc"] = build()
    return _STATE["nc"]


def run(x_q, x_kv, W_qkv, W_out, b_out, trace=False):
    nc = _get_nc()
    in_maps = make_in_maps(x_q, x_kv, W_qkv, W_out)
    res = run_bass_kernel_spmd(nc, in_maps, list(range(N_CORES)), trace=trace)
    out = combine([r["out"] for r in res.results], b_out)
    return out, res


def kernel(x_q, x_kv, W_qkv, W_out, b_out):
    out, _ = run(x_q, x_kv, W_qkv, W_out, b_out, trace=False)
    return out

